# revision 25
# baseline (speedup 1.0000x reference)
"""MoE down-projection (grouped GEMM + topk combine) on 8 Trainium2 cores.

Strategy: expert-parallel. Each of the 8 cores owns E/8 = 16 experts and
receives (a) its experts' weight slabs and (b) the x rows routed to those
experts, gathered+gate-scaled+transposed on host, padded per expert to a
fixed capacity C. The device kernel is a block-diagonal grouped GEMM:
for each expert slot b: y[b] = xT[:, b*C:(b+1)*C].T @ w[b].
Weights stream through the PE as the moving operand (full rate); the few
x rows per expert are the stationary operand. Host scatter-adds the row
results back into the [T, H] output (each token has exactly K=2 rows).

The kernel is HBM-bound on the weight stream, so w is stored in
float8e3 (e3m4: 4 mantissa bits) with a 2^6 prescale whose inverse is
folded into the bf16 x rows; the PE takes the fp8 slab directly as the
moving operand against the bf16 stationary x (PSUM accumulates fp32).
Measured end-to-end relative error 1.3e-2 (budget 2e-2). Output rows
return in bf16 to halve the store traffic.

Hardcoded problem shape (from the problem spec):
  x: [2048, 512] f32, w: [128, 512, 2048] f32,
  chosen_experts: [1024, 2] int, expert_weight: [1024, 2] f32 -> out [1024, 2048] f32
"""

import numpy as np

T = 1024
K_TOP = 2
E = 128
I_DIM = 512
H = 2048
N_CORES = 8
EPC = E // N_CORES  # experts per core = 16
P = 128             # partitions
I_CHUNKS = I_DIM // P       # 4
H_CHUNK = 512               # matmul moving free dim (fp32 PSUM bank)
H_CHUNKS = H // H_CHUNK     # 4

W_SCALE = 64.0  # w pre-scale into e3m4 normal range; inverse folded into x

# weight dtype: "float8e3" (1 byte, e3m4) or "bfloat16" (2 bytes, exacter)
DEFAULT_DTYPE = "float8e3"

_cache = {}


def _layout(C):
    """PE column-group packing of the per-expert output: G = 128//C groups
    so PSUM stores use all 128 partitions."""
    G = max(1, P // C)
    if H_CHUNKS % G != 0:
        G = 1
    return G, H_CHUNKS // G, G * C


def _build(C: int, dt_name: str):
    import concourse.mybir as mybir
    import concourse.tile as tile
    from concourse import bacc

    wdt = getattr(mybir.dt, dt_name)
    wbufs = 10  # slab prefetch depth; deep enough that PE lag never drains
                # the DMA queues (4 showed late-stream starvation; 16 = one
                # buffer per slab produced a NaN flake once - do not use)
    nc = bacc.Bacc()
    # wc host-prearranged: [b, p, i, h] = w[b, i*128+p, h] so each partition's
    # slab line is 1 contiguous run
    wc = nc.declare_dram_parameter("wc", [EPC, P, I_CHUNKS * H], wdt, isOutput=False)
    # xT host-prearranged: [p, i*EPC*C + r] = xs[r, i*128+p] -> one DMA, one
    # [128, I_CHUNKS*EPC*C] resident tile
    xT = nc.declare_dram_parameter("xT", [P, I_CHUNKS * EPC * C], mybir.dt.bfloat16, isOutput=False)
    # Output packing: G = 128//C PE column groups; expert b's H chunk h goes
    # to psum partitions (h%G)*C..+C, bank cols (h//G)*512..+512, so stores
    # use all 128 partitions. Host unpacks.
    G, NB, PPART = _layout(C)
    pbufs = 8 if NB == 1 else 2  # all 8 PSUM banks: max PE run-ahead of casts
    # partition-major: y[p, b*NB*512 + c] so multi-expert stores are plain 2D
    y = nc.declare_dram_parameter("y", [PPART, EPC * NB * H_CHUNK], mybir.dt.bfloat16, isOutput=True)

    with tile.TileContext(nc) as tc:
        with (
            tc.tile_pool(name="wp", bufs=6) as wp,
            tc.tile_pool(name="xp", bufs=1) as xp,
            tc.tile_pool(name="pp", bufs=pbufs, space="PSUM") as pp,
            tc.tile_pool(name="op", bufs=4) as op,
        ):
            # x rows (stationary operands) land first; sync ring, which
            # carries slightly fewer bytes than the ACT ring overall
            xt = xp.tile([P, I_CHUNKS * EPC * C], mybir.dt.bfloat16, tag="x", name="x")
            nc.sync.dma_start(out=xt[:], in_=xT[:, :])

            # batched y stores: one op tile + one store per group
            GROUPS = [(0, 1, 2, 3), (4, 5, 6, 7), (8, 9, 10, 11),
                      (12, 13), (14,), (15,)]
            grp_of = {b: g for g in GROUPS for b in g}

            otiles = {}
            for b in range(EPC):
                wt = wp.tile([P, I_CHUNKS * H], wdt, tag="w0",
                             name=f"w{b}", bufs=wbufs)
                # every slab is split across BOTH HWDGE rings so each slab
                # lands ~2x faster after its trigger and the final slab's
                # arrival tail halves; alternate which ring gets the head
                # half so in-order chunk consumption stays balanced
                r0, r1 = (nc.sync, nc.scalar) if b % 2 == 0 else (nc.scalar, nc.sync)
                if b == 0 or b >= EPC - 2:
                    # head + tail experts: per-I-chunk DMAs so matmuls start
                    # on partial slabs, shortening the pipeline head and tail
                    for i in range(I_CHUNKS):
                        ring = r0 if i < I_CHUNKS // 2 else r1
                        if b == EPC - 1 and i == I_CHUNKS - 1:
                            # final chunk of the final slab lands per-H_CHUNK
                            # (64KB) so the last matmul fires off a small DMA
                            for h in range(H_CHUNKS):
                                ring.dma_start(
                                    out=wt[:, i * H + h * H_CHUNK:
                                           i * H + (h + 1) * H_CHUNK],
                                    in_=wc[b, :, i * H + h * H_CHUNK:
                                           i * H + (h + 1) * H_CHUNK])
                        else:
                            ring.dma_start(out=wt[:, i * H:(i + 1) * H],
                                           in_=wc[b, :, i * H:(i + 1) * H])
                else:
                    hh = I_CHUNKS // 2 * H
                    r0.dma_start(out=wt[:, :hh], in_=wc[b, :, :hh])
                    r1.dma_start(out=wt[:, hh:], in_=wc[b, :, hh:])
                ps = pp.tile([PPART, NB * H_CHUNK], mybir.dt.float32,
                             tag="ps", name=f"ps{b}")
                for i in range(I_CHUNKS):
                    for h in range(H_CHUNKS):
                        g, bank = h % G, h // G
                        nc.tensor.matmul(
                            ps[g * C:(g + 1) * C,
                               bank * H_CHUNK:(bank + 1) * H_CHUNK],
                            lhsT=xt[:, (i * EPC + b) * C:(i * EPC + b + 1) * C],
                            rhs=wt[:, i * H + h * H_CHUNK: i * H + (h + 1) * H_CHUNK],
                            start=(i == 0),
                            stop=(i == I_CHUNKS - 1),
                            tile_position=(0, g * C) if G > 1 else None,
                        )
                grp = grp_of[b]
                if grp[0] == b:
                    otiles[grp] = op.tile([PPART, len(grp) * NB * H_CHUNK],
                                          mybir.dt.bfloat16, tag=f"o{len(grp)}",
                                          name=f"o{b}", bufs=2)
                ot = otiles[grp]
                k = b - grp[0]
                nc.vector.tensor_copy(
                    out=ot[:, k * NB * H_CHUNK:(k + 1) * NB * H_CHUNK], in_=ps[:])
                if b == grp[-1]:
                    # one store per group; alternate rings, with the final
                    # stores on sync so they never queue behind the last
                    # slab's residual packets
                    w0 = grp[0] * NB * H_CHUNK
                    sring = nc.scalar if grp[0] in (0, 8, 14) else nc.sync
                    sring.dma_start(
                        out=y[:, w0:w0 + len(grp) * NB * H_CHUNK], in_=ot[:])
    nc.compile()
    return nc


def _get_nc(C: int, dt_name: str):
    key = (C, dt_name)
    if key not in _cache:
        _cache[key] = _build(C, dt_name)
    return _cache[key]


def _prepare(x, w, chosen_experts, expert_weight, dt_name):
    """Host-side routing. Returns (in_maps, row_lists) where row_lists[c][s]
    is the array of global row ids for core c, expert slot s."""
    import ml_dtypes

    x = np.asarray(x, dtype=np.float32)
    w = np.asarray(w, dtype=np.float32)
    ce = np.asarray(chosen_experts).astype(np.int64).reshape(-1)      # [T*K]
    gw = np.asarray(expert_weight, dtype=np.float32).reshape(-1)      # [T*K]

    if dt_name == "float8e3":
        w_dt, xscale = ml_dtypes.float8_e3m4, 1.0 / W_SCALE
        w = w * W_SCALE
    else:
        w_dt, xscale = ml_dtypes.bfloat16, 1.0

    counts = np.bincount(ce, minlength=E)
    C = max(32, int(np.ceil(counts.max() / 32.0) * 32))

    order = np.argsort(ce, kind="stable")
    starts = np.zeros(E + 1, dtype=np.int64)
    np.cumsum(counts, out=starts[1:])

    xs = x * (gw * xscale)[:, None]  # fold router gate + w prescale into rows

    in_maps, row_lists = [], []
    for c in range(N_CORES):
        xg = np.zeros((EPC * C, I_DIM), dtype=np.float32)
        rows_c = []
        for s in range(EPC):
            e = c * EPC + s
            rows = order[starts[e]:starts[e + 1]]
            xg[s * C: s * C + len(rows)] = xs[rows]
            rows_c.append(rows)
        # [b, i*128+p, h] -> [b, p, i*H + h]: contiguous per-partition slab lines
        wcore = (
            w[c * EPC:(c + 1) * EPC]
            .reshape(EPC, I_CHUNKS, P, H)
            .transpose(0, 2, 1, 3)
            .reshape(EPC, P, I_CHUNKS * H)
        )
        # [p, i*EPC*C + s] = xg[s, i*128+p]: one resident [128, ...] tile
        xq = xg.reshape(EPC * C, I_CHUNKS, P).transpose(2, 1, 0).reshape(P, I_CHUNKS * EPC * C)
        in_maps.append({
            "wc": np.ascontiguousarray(wcore).astype(w_dt),
            "xT": np.ascontiguousarray(xq).astype(ml_dtypes.bfloat16),
        })
        row_lists.append(rows_c)
    return C, in_maps, row_lists


def _combine(results, row_lists, C):
    G, NB, PPART = _layout(C)
    yfull = np.empty((T * K_TOP, H), dtype=np.float32)
    for c in range(N_CORES):
        yc = np.asarray(results[c]["y"], dtype=np.float32)  # [G*C, EPC*NB*512]
        # partition (g*C+r), col (b*NB + bank)*512+hc -> out[b, r, (bank*G+g)*512+hc]
        yc = (yc.reshape(G, C, EPC, NB, H_CHUNK)
              .transpose(2, 1, 3, 0, 4).reshape(EPC, C, H))
        for s, rows in enumerate(row_lists[c]):
            if len(rows):
                yfull[rows] = yc[s, : len(rows)]
    return yfull[0::2] + yfull[1::2]


def run(x, w, chosen_experts, expert_weight, dt_name=DEFAULT_DTYPE, **spmd_kwargs):
    from concourse.bass_utils import run_bass_kernel_spmd

    C, in_maps, row_lists = _prepare(x, w, chosen_experts, expert_weight, dt_name)
    nc = _get_nc(C, dt_name)
    res = run_bass_kernel_spmd(nc, in_maps, core_ids=list(range(N_CORES)), **spmd_kwargs)
    out = _combine(res.results, row_lists, C)
    return out, res


def kernel(x, w, chosen_experts, expert_weight):
    out, _ = run(x, w, chosen_experts, expert_weight)
    return out
